# revision 1
# baseline (speedup 1.0000x reference)
"""Distributed Trainium2 (Bass/Tile) kernel for the KPCL contrastive loss.

Math (matches the jax reference):
  x1 = f + sign(f) * normalize(n1, 1e-8) * 0.1
  x2 = x1 + sign(x1) * normalize(n2, 1e-8) * 0.1
     = sign(f) * (|f| + 0.1*n1/max(||n1||,eps) + 0.1*n2/max(||n2||,eps))
  p  = relu(x2 @ W1 + b1) @ W2 + b2
  z  = p / max(||p||, 1e-6)
  sim = z @ z_all.T / T ;  lse_i = log(sum_j exp(sim_ij)) ; pos_i = sim_ii
  loss = mean(-pos + lse) + log(2)

Sharding: rows (N=8192) split across 8 cores, 1024 rows each. Each core
computes its z block in transposed layout zT [128, 1024], AllGathers zT
to [1024, 1024] (8 rank blocks of [128, 1024] = z_all^T), then computes
its row-block of sim as 128x512 matmuls (K=128 contraction) with a fused
exp+row-sum on the scalar engine. Per-core output is the scalar
sum_i(log(sumexp_i) - pos_i); the host sums, divides by N and adds log2.
"""

import sys

for _p in ("/opt/trn_rl_repo",):
    if _p not in sys.path:
        sys.path.append(_p)

import numpy as np

import concourse.bass as bass
import concourse.tile as tile
from concourse import mybir
from concourse.bass_utils import run_bass_kernel_spmd
from concourse.masks import make_identity

F32 = mybir.dt.float32
BF16 = mybir.dt.bfloat16
U32 = mybir.dt.uint32

N_CORES = 8
N = 8192
ROWS = N // N_CORES          # 1024 rows per core
D_IN = 512
D_PROJ = 128
TEMP = 0.15
P = 128                      # partitions
NBLK = ROWS // P             # 8 row-blocks per core
INV_T = 1.0 / TEMP

AF = mybir.ActivationFunctionType
OP = mybir.AluOpType


def split_excess_waits(nc: bass.Bass, max_waits: int = 1) -> int:
    """Hoist excess sem waits onto same-engine nop carriers.

    The walrus build in this image rejects instructions carrying more
    than ~2 sync commands ("Too many sync wait commands"), but Tile's
    wait assignment freely emits 2-3 waits per instruction. Splitting
    the waits onto preceding nop instructions on the same engine queue
    is semantically identical (engine program order is preserved).
    """
    nmoved = 0
    for f in nc.m.functions:
        for b in f.blocks:
            il = b.instructions
            i = 0
            while i < len(il):
                inst = il[i]
                si = inst.sync_info
                if si is None or not si.on_wait or len(si.on_wait) <= max_waits:
                    i += 1
                    continue
                eng = inst.engine
                if eng is None:
                    i += 1
                    continue
                waits = list(si.on_wait)
                keep = waits[-max_waits:]
                excess = waits[:-max_waits]
                carriers = []
                for w in excess:
                    nop = nc.engines[eng].nop().ins
                    for f2 in nc.m.functions:
                        for b2 in f2.blocks:
                            try:
                                b2.instructions.remove(nop)
                            except ValueError:
                                pass
                    nop.sync_info = mybir.SyncInfo(on_wait=[w], on_update=[])
                    carriers.append(nop)
                inst.sync_info = mybir.SyncInfo(on_wait=keep,
                                                on_update=list(si.on_update))
                for c in reversed(carriers):
                    il.insert(i, c)
                i += 1 + len(carriers)
                nmoved += len(excess)
    return nmoved


def build_nc(phase: str = "full") -> bass.Bass:
    # phase: "A" (local z only), "AG" (+allgather+loads), "full"
    nc = bass.Bass("TRN2", target_bir_lowering=False, debug=False,
                   num_devices=N_CORES)

    f_d = nc.dram_tensor("features", [ROWS, D_IN], F32, kind="ExternalInput")
    u1_d = nc.dram_tensor("noise1", [ROWS, D_IN], F32, kind="ExternalInput")
    u2_d = nc.dram_tensor("noise2", [ROWS, D_IN], F32, kind="ExternalInput")
    w1_d = nc.dram_tensor("W1", [D_IN, D_PROJ], F32, kind="ExternalInput")
    b1_d = nc.dram_tensor("b1", [D_PROJ, 1], F32, kind="ExternalInput")
    w2_d = nc.dram_tensor("W2", [D_PROJ, D_PROJ], F32, kind="ExternalInput")
    b2_d = nc.dram_tensor("b2", [D_PROJ, 1], F32, kind="ExternalInput")
    out_d = nc.dram_tensor("out", [1, 1], F32, kind="ExternalOutput")

    # collective bounce buffers (internal DRAM; AG output must be Shared)
    zT_bounce = nc.dram_tensor("zT_bounce", [P, ROWS], F32)
    zall_bounce = nc.dram_tensor("zall_bounce", [N_CORES * P, ROWS], F32,
                                 addr_space="Shared")

    with tile.TileContext(nc) as tc:
        with (
            tc.tile_pool(name="singles", bufs=1) as singles,
            tc.tile_pool(name="work", bufs=3) as work,
            tc.tile_pool(name="small", bufs=3) as small,
            tc.tile_pool(name="expsc", bufs=2) as expsc,
        ):
            # ---- constants / persistent tiles ----
            w1t = singles.tile([P, 4, P], F32)      # W1 k-chunks (lhsT)
            for c in range(4):
                nc.sync.dma_start(w1t[:, c, :], w1_d[c * P:(c + 1) * P, :])
            w2t = singles.tile([P, P], F32)
            nc.sync.dma_start(w2t[:], w2_d[:, :])
            b1t = singles.tile([P, 1], F32)
            nc.sync.dma_start(b1t[:], b1_d[:, :])
            b2t = singles.tile([P, 1], F32)
            nc.sync.dma_start(b2t[:], b2_d[:, :])

            ident = singles.tile([P, P], F32)
            make_identity(nc, ident[:])
            ones_col = singles.tile([P, 1], F32)
            nc.gpsimd.memset(ones_col[:], 1.0)
            ones_row = singles.tile([1, P], F32)
            nc.gpsimd.memset(ones_row[:], 1.0)
            zbias = singles.tile([P, 1], F32)
            nc.gpsimd.memset(zbias[:], 0.0)
            zbias1 = singles.tile([1, 1], F32)
            nc.gpsimd.memset(zbias1[:], 0.0)

            zT = singles.tile([P, ROWS], F32)       # z^T for this core
            logS = singles.tile([P, NBLK], F32)     # log(sumexp) per block
            pos_all = singles.tile([1, ROWS], F32)  # diag(sim) per local row
            zallT = singles.tile([P, N_CORES, ROWS], F32)  # gathered z_all^T

            # =========== Phase A: augment + projection + normalize ==========
            with (
                tc.tile_pool(name="psA2", bufs=2, space="PSUM") as psA2,
                tc.tile_pool(name="psA1", bufs=1, space="PSUM") as psA1,
            ):
                for m in range(NBLK):
                    rs = slice(m * P, (m + 1) * P)
                    ft = work.tile([P, D_IN], F32, tag="F")
                    nc.sync.dma_start(ft[:], f_d[rs, :])
                    u1 = work.tile([P, D_IN], F32, tag="U1")
                    nc.sync.dma_start(u1[:], u1_d[rs, :])
                    u2 = work.tile([P, D_IN], F32, tag="U2")
                    nc.sync.dma_start(u2[:], u2_d[rs, :])

                    # noise norms: s = sum(u^2); r = 0.1/max(sqrt(s), 1e-8)
                    sq = work.tile([P, D_IN], F32, tag="sq")
                    s1 = small.tile([P, 1], F32, tag="s1")
                    nc.vector.scalar_tensor_tensor(
                        out=sq[:], in0=u1[:], scalar=1.0, in1=u1[:],
                        op0=OP.mult, op1=OP.mult, accum_out=s1[:])
                    sq2 = work.tile([P, D_IN], F32, tag="sq")
                    s2 = small.tile([P, 1], F32, tag="s2")
                    nc.vector.scalar_tensor_tensor(
                        out=sq2[:], in0=u2[:], scalar=1.0, in1=u2[:],
                        op0=OP.mult, op1=OP.mult, accum_out=s2[:])

                    n1 = small.tile([P, 1], F32, tag="n1")
                    nc.scalar.activation(n1[:], s1[:], AF.Sqrt, bias=zbias[:])
                    n2 = small.tile([P, 1], F32, tag="n2")
                    nc.scalar.activation(n2[:], s2[:], AF.Sqrt, bias=zbias[:])
                    # rN = 1 / (10 * max(n, 1e-8))  == 0.1 / max(n, 1e-8)
                    n1c = small.tile([P, 1], F32, tag="n1c")
                    nc.vector.tensor_scalar(out=n1c[:], in0=n1[:], scalar1=1e-8,
                                            scalar2=10.0, op0=OP.max, op1=OP.mult)
                    r1 = small.tile([P, 1], F32, tag="r1")
                    nc.vector.reciprocal(r1[:], n1c[:])
                    n2c = small.tile([P, 1], F32, tag="n2c")
                    nc.vector.tensor_scalar(out=n2c[:], in0=n2[:], scalar1=1e-8,
                                            scalar2=10.0, op0=OP.max, op1=OP.mult)
                    r2 = small.tile([P, 1], F32, tag="r2")
                    nc.vector.reciprocal(r2[:], n2c[:])

                    # |f| and sign bit
                    absf = work.tile([P, D_IN], F32, tag="absf")
                    nc.vector.tensor_scalar(
                        out=absf[:].bitcast(U32), in0=ft[:].bitcast(U32),
                        scalar1=0x7FFFFFFF, scalar2=None, op0=OP.bitwise_and)
                    sgn = work.tile([P, D_IN], F32, tag="sgn")
                    nc.vector.tensor_scalar(
                        out=sgn[:].bitcast(U32), in0=ft[:].bitcast(U32),
                        scalar1=0x80000000, scalar2=None, op0=OP.bitwise_and)

                    # a = |f| + u1*r1 + u2*r2 ; x2 = a | signbit
                    bt = work.tile([P, D_IN], F32, tag="bt")
                    nc.vector.scalar_tensor_tensor(
                        out=bt[:], in0=u1[:], scalar=r1[:], in1=absf[:],
                        op0=OP.mult, op1=OP.add)
                    at = work.tile([P, D_IN], F32, tag="at")
                    nc.vector.scalar_tensor_tensor(
                        out=at[:], in0=u2[:], scalar=r2[:], in1=bt[:],
                        op0=OP.mult, op1=OP.add)
                    x2 = work.tile([P, D_IN], F32, tag="x2")
                    nc.vector.tensor_tensor(
                        out=x2[:].bitcast(U32), in0=at[:].bitcast(U32),
                        in1=sgn[:].bitcast(U32), op=OP.bitwise_or)

                    # transpose x2 into [512part-chunks, 128rows]
                    xT = work.tile([P, 4, P], F32, tag="xT")
                    for c in range(4):
                        tp = psA2.tile([P, P], F32, tag="tp")
                        nc.tensor.transpose(tp[:], x2[:, c * P:(c + 1) * P],
                                            ident[:])
                        nc.any.tensor_copy(xT[:, c, :], tp[:])

                    # hT = relu(W1^T-chunks contraction + b1)
                    hps = psA2.tile([P, P], F32, tag="hT")
                    for c in range(4):
                        nc.tensor.matmul(hps[:], w1t[:, c, :], xT[:, c, :],
                                         start=(c == 0), stop=(c == 3))
                    hT = work.tile([P, P], F32, tag="hT_sb")
                    nc.scalar.activation(hT[:], hps[:], AF.Relu, bias=b1t[:])

                    # pT = W2^T @ hT + b2
                    pps = psA1.tile([P, P], F32, tag="pT")
                    nc.tensor.matmul(pps[:], w2t[:], hT[:])
                    pT = work.tile([P, P], F32, tag="pT_sb")
                    nc.scalar.activation(pT[:], pps[:], AF.Identity, bias=b2t[:])

                    # row sumsq via ones-matmul (partition-axis reduction)
                    sqp = work.tile([P, P], F32, tag="sqp")
                    nc.vector.tensor_tensor(out=sqp[:], in0=pT[:], in1=pT[:],
                                            op=OP.mult)
                    nsq = psA1.tile([1, P], F32, tag="nsq")
                    nc.tensor.matmul(nsq[:], ones_col[:], sqp[:])

                    # norm with one Newton step on sqrt, then clamp+recip
                    n0 = small.tile([1, P], F32, tag="n0")
                    nc.scalar.activation(n0[:], nsq[:], AF.Sqrt, bias=zbias1[:])
                    t0 = small.tile([1, P], F32, tag="t0")
                    nc.vector.reciprocal(t0[:], n0[:])
                    th = small.tile([1, P], F32, tag="th")
                    nc.vector.tensor_tensor(out=th[:], in0=t0[:], in1=nsq[:],
                                            op=OP.mult)
                    th2 = small.tile([1, P], F32, tag="th2")
                    nc.vector.tensor_tensor(out=th2[:], in0=th[:], in1=n0[:],
                                            op=OP.add)
                    ncl = small.tile([1, P], F32, tag="ncl")
                    nc.vector.tensor_scalar(out=ncl[:], in0=th2[:], scalar1=0.5,
                                            scalar2=1e-6, op0=OP.mult, op1=OP.max)
                    rsz = small.tile([1, P], F32, tag="rsz")
                    nc.vector.reciprocal(rsz[:], ncl[:])

                    # broadcast rsz across partitions via K=1 matmul
                    bc = psA1.tile([P, P], F32, tag="bc")
                    nc.tensor.matmul(bc[:], ones_row[:], rsz[:])
                    nc.vector.tensor_tensor(out=zT[:, rs], in0=pT[:], in1=bc[:],
                                            op=OP.mult)

                    # pos = nsq * rsz^2 / T   (diag of sim for these rows)
                    tmp2 = small.tile([1, P], F32, tag="tmp2")
                    nc.vector.tensor_tensor(out=tmp2[:], in0=nsq[:], in1=rsz[:],
                                            op=OP.mult)
                    nc.vector.scalar_tensor_tensor(
                        out=pos_all[:, rs], in0=tmp2[:], scalar=INV_T,
                        in1=rsz[:], op0=OP.mult, op1=OP.mult)

            if phase == "A":
                nc.sync.dma_start(out=out_d[:, :], in_=zT[0:1, 0:1])

            if phase in ("AG", "full"):
                # =============== AllGather z^T across cores =================
                nc.sync.dma_start(out=zT_bounce[:, :], in_=zT[:])
                nc.gpsimd.collective_compute(
                    "AllGather",
                    OP.bypass,
                    ins=[zT_bounce[:, :]],
                    outs=[zall_bounce[:, :]],
                    replica_groups=[list(range(N_CORES))],
                )
                for r in range(N_CORES):
                    nc.sync.dma_start(out=zallT[:, r, :],
                                      in_=zall_bounce[r * P:(r + 1) * P, :])

            if phase == "AG":
                nc.sync.dma_start(out=out_d[:, :], in_=zallT[0:1, 0, 0:1])

            if phase == "full":
                # ======== Phase C: sim row-block + fused exp/rowsum =========
                with tc.tile_pool(name="psC", bufs=2, space="PSUM") as psC:
                    for m in range(NBLK):
                        lhsT = zT[:, m * P:(m + 1) * P]
                        sacc = small.tile([P, 4], F32, tag="sacc")
                        for g in range(4):
                            ps = psC.tile([P, 4, 512], F32, tag="sim")
                            for j in range(4):
                                col = g * 2048 + j * 512
                                r, off = divmod(col, ROWS)
                                nc.tensor.matmul(ps[:, j, :], lhsT,
                                                 zallT[:, r, off:off + 512])
                            sc = expsc.tile([P, 4, 512], F32, tag="expout")
                            nc.scalar.activation(sc[:], ps[:], AF.Exp,
                                                 bias=zbias[:], scale=INV_T,
                                                 accum_out=sacc[:, g:g + 1])
                        S = small.tile([P, 1], F32, tag="S")
                        nc.vector.tensor_reduce(out=S[:], in_=sacc[:],
                                                axis=mybir.AxisListType.X,
                                                op=OP.add)
                        nc.scalar.activation(logS[:, m:m + 1], S[:], AF.Ln,
                                             bias=zbias[:])

                    # final local reduction: out = sum(logS) - sum(pos)
                    possum = small.tile([1, 1], F32, tag="possum")
                    nc.vector.tensor_reduce(out=possum[:], in_=pos_all[:],
                                            axis=mybir.AxisListType.X,
                                            op=OP.add)
                    lps = psC.tile([1, NBLK], F32, tag="sim")
                    nc.tensor.matmul(lps[:], ones_col[:], logS[:])
                    lsum = small.tile([1, 1], F32, tag="lsum")
                    nc.vector.tensor_reduce(out=lsum[:], in_=lps[:],
                                            axis=mybir.AxisListType.X,
                                            op=OP.add)
                    res = small.tile([1, 1], F32, tag="res")
                    nc.vector.tensor_tensor(out=res[:], in0=lsum[:],
                                            in1=possum[:], op=OP.subtract)
                    nc.sync.dma_start(out=out_d[:, :], in_=res[:])

    split_excess_waits(nc)
    return nc


_NC_CACHE = None


def _get_nc():
    global _NC_CACHE
    if _NC_CACHE is None:
        _NC_CACHE = build_nc()
    return _NC_CACHE


def run_spmd(inputs, trace=False, **kw):
    feats = np.ascontiguousarray(inputs["features"], dtype=np.float32)
    n1 = np.ascontiguousarray(inputs["noise1"], dtype=np.float32)
    n2 = np.ascontiguousarray(inputs["noise2"], dtype=np.float32)
    w1 = np.ascontiguousarray(inputs["W1"], dtype=np.float32)
    b1 = np.ascontiguousarray(inputs["b1"], dtype=np.float32).reshape(D_PROJ, 1)
    w2 = np.ascontiguousarray(inputs["W2"], dtype=np.float32)
    b2 = np.ascontiguousarray(inputs["b2"], dtype=np.float32).reshape(D_PROJ, 1)

    in_maps = []
    for r in range(N_CORES):
        sl = slice(r * ROWS, (r + 1) * ROWS)
        in_maps.append({
            "features": feats[sl], "noise1": n1[sl], "noise2": n2[sl],
            "W1": w1, "b1": b1, "W2": w2, "b2": b2,
        })
    nc = _get_nc()
    return run_bass_kernel_spmd(nc, in_maps, core_ids=list(range(N_CORES)),
                                trace=trace, **kw)


def kernel(**inputs) -> np.ndarray:
    out = run_spmd(inputs)
    total = sum(float(out.results[r]["out"][0, 0]) for r in range(N_CORES))
    loss = total / float(N) + float(np.log(np.float32(2.0)))
    return np.array(loss, dtype=np.float32)



# revision 22
# speedup vs baseline: 1.1279x; 1.1279x over previous
"""Distributed Trainium2 (Bass/Tile) kernel for the KPCL contrastive loss.

Math (matches the jax reference):
  x1 = f + sign(f) * normalize(n1, 1e-8) * 0.1
  x2 = x1 + sign(x1) * normalize(n2, 1e-8) * 0.1
     = sign(f) * (|f| + 0.1*n1/||n1|| + 0.1*n2/||n2||)
  p  = relu(x2 @ W1 + b1) @ W2 + b2
  z  = p / ||p||                          (so diag(sim) == 1/T exactly)
  sim = z @ z_all.T / T ;  lse_i = log(sum_j exp(sim_ij))
  loss = mean(lse) - 1/T + log(2)

Sharding: rows (N=8192) split across 8 cores, 1024 rows each. Each core
computes its z block in transposed layout zT [128, 1024] (bf16), AllGathers
zT to [1024, 1024] bf16, then computes its row-block of sim as bf16 128x512
matmuls (K=128 contraction). The exp+row-sum over the [1024, 8192] sim block
is split across three engines: the scalar engine runs native Exp with
accumulate, while DVE and GpSimd run a Schraudolph-style exp (bf16 bit
pattern = trunc(sim*1231.07 + 16249)) followed by an accumulate pass.
Per-core output is the scalar sum_i log(sumexp_i); the host computes
loss = sum/N - 1/T + log 2.
"""

import sys

for _p in ("/opt/trn_rl_repo",):
    if _p not in sys.path:
        sys.path.append(_p)

import numpy as np

import concourse.bass as bass
import concourse.tile as tile
from concourse import mybir
from concourse.bass_utils import run_bass_kernel_spmd
from concourse.masks import make_identity

F32 = mybir.dt.float32
F32R = mybir.dt.float32r
BF16 = mybir.dt.bfloat16
U16 = mybir.dt.uint16
U32 = mybir.dt.uint32

N_CORES = 8
N = 8192
ROWS = N // N_CORES          # 1024 rows per core
D_IN = 512
D_PROJ = 128
TEMP = 0.15
P = 128                      # partitions
NBLK = ROWS // P             # 8 row-blocks per core
INV_T = 1.0 / TEMP

# Schraudolph bf16-bit exp: bits16(e^x) ~= trunc(x * 128/ln2 + (127*128 - C))
SCH_A = 128.0 / np.log(2.0)          # 184.664965
SCH_C = 7.0                          # tuned for zero-mean sum error (trunc)
SCH_B = 127.0 * 128.0 - SCH_C

AF = mybir.ActivationFunctionType
OP = mybir.AluOpType


def split_excess_waits(nc: bass.Bass, max_waits: int = 1) -> int:
    """Hoist excess sem waits onto same-engine nop carriers.

    The walrus build in this image rejects instructions carrying more
    than ~2 sync commands ("Too many sync wait commands"), but Tile's
    wait assignment freely emits 2-3 waits per instruction. Splitting
    the waits onto preceding nop instructions on the same engine queue
    is semantically identical (engine program order is preserved).
    """
    nmoved = 0
    for f in nc.m.functions:
        for b in f.blocks:
            il = b.instructions
            i = 0
            while i < len(il):
                inst = il[i]
                si = inst.sync_info
                if si is None or not si.on_wait or len(si.on_wait) <= max_waits:
                    i += 1
                    continue
                eng = inst.engine
                if eng is None:
                    i += 1
                    continue
                waits = list(si.on_wait)
                keep = waits[-max_waits:]
                excess = waits[:-max_waits]
                carriers = []
                for w in excess:
                    nop = nc.engines[eng].nop().ins
                    for f2 in nc.m.functions:
                        for b2 in f2.blocks:
                            try:
                                b2.instructions.remove(nop)
                            except ValueError:
                                pass
                    nop.sync_info = mybir.SyncInfo(on_wait=[w], on_update=[])
                    carriers.append(nop)
                inst.sync_info = mybir.SyncInfo(on_wait=keep,
                                                on_update=list(si.on_update))
                for c in reversed(carriers):
                    il.insert(i, c)
                i += 1 + len(carriers)
                nmoved += len(excess)
    return nmoved


def build_nc(phase: str = "full") -> bass.Bass:
    # phase: "A" (local z only), "AG" (+allgather+loads), "full"
    nc = bass.Bass("TRN2", target_bir_lowering=False, debug=False,
                   num_devices=N_CORES)

    f_d = nc.dram_tensor("features", [ROWS, D_IN], F32, kind="ExternalInput")
    u1_d = nc.dram_tensor("noise1", [ROWS, D_IN], F32, kind="ExternalInput")
    u2_d = nc.dram_tensor("noise2", [ROWS, D_IN], F32, kind="ExternalInput")
    w1_d = nc.dram_tensor("W1", [D_IN, D_PROJ], F32, kind="ExternalInput")
    b1_d = nc.dram_tensor("b1", [D_PROJ, 1], F32, kind="ExternalInput")
    w2_d = nc.dram_tensor("W2", [D_PROJ, D_PROJ], F32, kind="ExternalInput")
    b2_d = nc.dram_tensor("b2", [D_PROJ, 1], F32, kind="ExternalInput")
    out_d = nc.dram_tensor("out", [1, 1], F32, kind="ExternalOutput")

    # collective bounce buffers (internal DRAM; AG output must be Shared)
    zT_bounce = nc.dram_tensor("zT_bounce", [P, ROWS], BF16)
    zall_bounce = nc.dram_tensor("zall_bounce", [N_CORES * P, ROWS], BF16,
                                 addr_space="Shared")

    GROUPS = 2                   # phase-A row groups
    GR = ROWS // GROUPS          # 512 rows per group
    SUB = GR // P                # 4 subtiles of 128 rows per group
    LN01 = float(np.log(0.1))    # bias so exp(-0.5*ln(s) + LN01) = 0.1/sqrt(s)

    with tile.TileContext(nc) as tc:
        with (
            tc.tile_pool(name="singles", bufs=1) as singles,
            tc.tile_pool(name="work", bufs=3) as work,
            tc.tile_pool(name="small", bufs=4) as small,
            tc.tile_pool(name="expsc", bufs=3) as expsc,
        ):
            # ---- constants / persistent tiles ----
            w1t = singles.tile([P, 4, P], F32)      # W1 k-chunks (lhsT)
            for c in range(4):
                nc.sync.dma_start(w1t[:, c, :], w1_d[c * P:(c + 1) * P, :])
            w2t = singles.tile([P, P], F32)
            nc.sync.dma_start(w2t[:], w2_d[:, :])
            b1t = singles.tile([P, 1], F32)
            nc.sync.dma_start(b1t[:], b1_d[:, :])
            b2t = singles.tile([P, 1], F32)
            nc.sync.dma_start(b2t[:], b2_d[:, :])

            ident = singles.tile([P, P], BF16)
            make_identity(nc, ident[:])
            ones_col = singles.tile([P, 1], F32)
            nc.gpsimd.memset(ones_col[:], 1.0)
            ones_col_b = singles.tile([P, 1], BF16)
            nc.gpsimd.memset(ones_col_b[:], 1.0)
            ones_row = singles.tile([1, P], F32)
            nc.gpsimd.memset(ones_row[:], 1.0)
            zbias = singles.tile([P, 1], F32)
            nc.gpsimd.memset(zbias[:], 0.0)
            ln01b = singles.tile([P, 1], F32)
            nc.gpsimd.memset(ln01b[:], float(np.log(0.1)))

            # bf16 copies of the weights for 1-cycle/row matmuls
            w1b = singles.tile([P, 4, P], BF16)
            nc.vector.tensor_copy(w1b[:], w1t[:])
            w2b = singles.tile([P, P], BF16)
            nc.vector.tensor_copy(w2b[:], w2t[:])

            zTb = singles.tile([P, ROWS], BF16)     # z^T (bf16) for this core
            logS = singles.tile([P, NBLK], F32)     # log(sumexp) per block
            zallTb = singles.tile([P, N_CORES, ROWS], BF16)  # gathered z^T

            # =========== Phase A: augment + projection + normalize ==========
            # x2 = f + sign(f) * (u1*r1 + u2*r2), r = 0.1/||u||
            # Engine split per subtile: scalar{sq1+accum, sign}; DVE{sq2+accum,
            # t1, t2, xT copy}; gpsimd{t*s mult, +f add}; PE{transposes}.
            # All activation funcs live in the natural_log_exp table (no
            # sqrt!): 0.1/sqrt(s) = exp(-0.5*ln(s) + ln(0.1)).
            with (
                tc.tile_pool(name="psA2", bufs=2, space="PSUM") as psA2,
                tc.tile_pool(name="psA1", bufs=1, space="PSUM") as psA1,
            ):
                for g in range(GROUPS):
                    xT = work.tile([P, 4, GR], BF16, tag="xT")
                    for half in range(SUB // 2):
                      subs = [2 * half, 2 * half + 1]
                      s1g = small.tile([P, 2], F32, tag="s1g")
                      s2g = small.tile([P, 2], F32, tag="s2g")
                      x2bs = []
                      for i, s in enumerate(subs):
                        rs = slice(g * GR + s * P, g * GR + (s + 1) * P)
                        ft = work.tile([P, D_IN], F32, tag="F")
                        nc.sync.dma_start(ft[:], f_d[rs, :])
                        u1 = work.tile([P, D_IN], F32, tag="U1")
                        nc.sync.dma_start(u1[:], u1_d[rs, :])
                        u2 = work.tile([P, D_IN], F32, tag="U2")
                        nc.sync.dma_start(u2[:], u2_d[rs, :])

                        # noise sumsq: s1 on scalar engine, s2 on DVE
                        sq1 = work.tile([P, D_IN], F32, tag="sq1")
                        nc.scalar.activation(sq1[:], u1[:], AF.Square,
                                             accum_out=s1g[:, i:i + 1])
                        sq2 = work.tile([P, D_IN], F32, tag="sq2")
                        nc.vector.scalar_tensor_tensor(
                            out=sq2[:], in0=u2[:], scalar=1.0, in1=u2[:],
                            op0=OP.mult, op1=OP.mult,
                            accum_out=s2g[:, i:i + 1])

                        # sign(f) as +-1/0 floats (scalar engine)
                        sgn = work.tile([P, D_IN], F32, tag="sgn")
                        nc.scalar.activation(sgn[:], ft[:], AF.Sign)
                        x2bs.append((ft, u1, u2, sgn))

                      # r1/r2 for both subtiles in two ln+exp pairs
                      l1 = small.tile([P, 2], F32, tag="l1")
                      nc.scalar.activation(l1[:], s1g[:], AF.Ln, bias=zbias[:])
                      r1g = small.tile([P, 2], F32, tag="r1g")
                      nc.scalar.activation(r1g[:], l1[:], AF.Exp, scale=-0.5,
                                           bias=ln01b[:])
                      l2 = small.tile([P, 2], F32, tag="l2")
                      nc.scalar.activation(l2[:], s2g[:], AF.Ln, bias=zbias[:])
                      r2g = small.tile([P, 2], F32, tag="r2g")
                      nc.scalar.activation(r2g[:], l2[:], AF.Exp, scale=-0.5,
                                           bias=ln01b[:])

                      for i, s in enumerate(subs):
                        ft, u1, u2, sgn = x2bs[i]
                        # t = u1*r1 + u2*r2 (DVE), then x2 = f + sign(f)*t
                        # (gpsimd, float ops only)
                        t1 = work.tile([P, D_IN], F32, tag="t1")
                        nc.vector.tensor_scalar(
                            out=t1[:], in0=u1[:], scalar1=r1g[:, i:i + 1],
                            scalar2=None, op0=OP.mult)
                        t2 = work.tile([P, D_IN], F32, tag="t2")
                        nc.vector.scalar_tensor_tensor(
                            out=t2[:], in0=u2[:], scalar=r2g[:, i:i + 1],
                            in1=t1[:], op0=OP.mult, op1=OP.add)
                        st = work.tile([P, D_IN], F32, tag="st")
                        nc.gpsimd.tensor_tensor(out=st[:], in0=t2[:],
                                                in1=sgn[:], op=OP.mult)
                        x2b = work.tile([P, D_IN], BF16, tag="x2b")
                        nc.gpsimd.tensor_tensor(out=x2b[:], in0=st[:],
                                                in1=ft[:], op=OP.add)

                        # transpose x2b into xT[:, c, s*128:(s+1)*128]
                        tpp = psA2.tile([P, 4, P], BF16, tag="tp")
                        for c in range(4):
                            nc.tensor.transpose(tpp[:, c, :],
                                                x2b[:, c * P:(c + 1) * P],
                                                ident[:])
                        nc.vector.tensor_copy(xT[:, :, s * P:(s + 1) * P],
                                              tpp[:])

                    # hT = relu(W1^T-chunks contraction + b1)  [128, 512]
                    hps = psA1.tile([P, GR], F32, tag="hT")
                    for c in range(4):
                        nc.tensor.matmul(hps[:], w1b[:, c, :], xT[:, c, :],
                                         start=(c == 0), stop=(c == 3))
                    hT = work.tile([P, GR], BF16, tag="hT_sb")
                    nc.scalar.activation(hT[:], hps[:], AF.Relu, bias=b1t[:])

                    # pT = W2^T @ hT + b2
                    pps = psA1.tile([P, GR], F32, tag="pT")
                    nc.tensor.matmul(pps[:], w2b[:], hT[:])
                    pT = work.tile([P, GR], F32, tag="pT_sb")
                    nc.scalar.activation(pT[:], pps[:], AF.Identity,
                                         bias=b2t[:])

                    # row norms: sumsq via ones-matmul (partition reduction)
                    sqp = work.tile([P, GR], BF16, tag="sqp")
                    nc.scalar.activation(sqp[:], pT[:], AF.Square)
                    nsq = psA1.tile([1, GR], F32, tag="nsq")
                    nc.tensor.matmul(nsq[:], ones_col_b[:], sqp[:])
                    # rsz = 1/sqrt(nsq) = exp(-0.5*ln(nsq)), scalar engine
                    lnn = small.tile([1, GR], F32, tag="lnn")
                    nc.scalar.activation(lnn[:], nsq[:], AF.Ln,
                                         bias=zbias[0:1, :])
                    rsz = small.tile([1, GR], F32, tag="rsz")
                    nc.scalar.activation(rsz[:], lnn[:], AF.Exp, scale=-0.5,
                                         bias=zbias[0:1, :])

                    # broadcast rsz across partitions via K=1 matmul
                    bc = psA1.tile([P, GR], F32, tag="bc")
                    nc.tensor.matmul(bc[:], ones_row[:], rsz[:])
                    nc.vector.tensor_tensor(
                        out=zTb[:, g * GR:(g + 1) * GR], in0=pT[:], in1=bc[:],
                        op=OP.mult)

            if phase == "A":
                dbg = small.tile([1, 1], F32, tag="dbg")
                nc.vector.tensor_copy(dbg[:], zTb[0:1, 0:1])
                nc.sync.dma_start(out=out_d[:, :], in_=dbg[:])

            if phase in ("AG", "full"):
                # =============== AllGather z^T (bf16) across cores ==========
                nc.sync.dma_start(out=zT_bounce[:, :], in_=zTb[:])
                nc.gpsimd.collective_compute(
                    "AllGather",
                    OP.bypass,
                    ins=[zT_bounce[:, :]],
                    outs=[zall_bounce[:, :]],
                    replica_groups=[list(range(N_CORES))],
                )
                for r in range(N_CORES):
                    nc.sync.dma_start(out=zallTb[:, r, :],
                                      in_=zall_bounce[r * P:(r + 1) * P, :])

            if phase == "AG":
                dbg = small.tile([1, 1], F32, tag="dbg")
                nc.vector.tensor_copy(dbg[:], zallTb[0:1, 0, 0:1])
                nc.sync.dma_start(out=out_d[:, :], in_=dbg[:])

            if phase == "full":
                # ======== Phase C: sim row-block + 2-engine exp/rowsum ======
                # 4 chunks of 2048 columns per block; scalar does Exp+accum
                # on ~2.5 chunks, DVE schraudolph on ~1.5.
                with tc.tile_pool(name="psC", bufs=2, space="PSUM") as psC:
                    for m in range(NBLK):
                        lhsT = zTb[:, m * P:(m + 1) * P]
                        n_s = 3 if m % 2 else 2   # chunks on scalar engine
                        sacc = small.tile([P, 4], F32, tag="sacc")
                        for ch in range(4):
                            ps = psC.tile([P, 4, 512], F32, tag="sim")
                            for j in range(4):
                                col = ch * 2048 + j * 512
                                r, off = divmod(col, ROWS)
                                nc.tensor.matmul(
                                    ps[:, j, :], lhsT,
                                    zallTb[:, r, off:off + 512])
                            acc = sacc[:, ch:ch + 1]
                            if ch < n_s:
                                sc = expsc.tile([P, 4, 512], BF16, tag="esc")
                                nc.scalar.activation(sc[:], ps[:], AF.Exp,
                                                     bias=zbias[:],
                                                     scale=INV_T,
                                                     accum_out=acc)
                            else:
                                # schraudolph: u16 = trunc(sim/T * A + B) are
                                # the bf16 bits of exp(sim/T)
                                u16t = expsc.tile([P, 4, 512], U16, tag="u16")
                                nc.vector.tensor_scalar(
                                    out=u16t[:], in0=ps[:],
                                    scalar1=float(SCH_A * INV_T),
                                    scalar2=float(SCH_B),
                                    op0=OP.mult, op1=OP.add)
                                dummy = expsc.tile([P, 4, 512], BF16,
                                                   tag="dm")
                                nc.vector.tensor_scalar(
                                    out=dummy[:], in0=u16t[:].bitcast(BF16),
                                    scalar1=1.0, scalar2=0.0, op0=OP.mult,
                                    op1=OP.add, accum_out=acc)
                        S = small.tile([P, 1], F32, tag="S")
                        nc.vector.tensor_reduce(out=S[:], in_=sacc[:],
                                                axis=mybir.AxisListType.X,
                                                op=OP.add)
                        nc.scalar.activation(logS[:, m:m + 1], S[:], AF.Ln,
                                             bias=zbias[:])

                # final local reduction: out = sum(logS)
                with tc.tile_pool(name="psF", bufs=1, space="PSUM") as psF:
                    lsum = small.tile([P, 1], F32, tag="lsum")
                    nc.vector.tensor_reduce(out=lsum[:], in_=logS[:],
                                            axis=mybir.AxisListType.X,
                                            op=OP.add)
                    lps = psF.tile([1, 1], F32, tag="lps")
                    nc.tensor.matmul(lps[:], ones_col[:], lsum[:])
                    res = small.tile([1, 1], F32, tag="res")
                    nc.vector.tensor_copy(res[:], lps[:])
                    nc.sync.dma_start(out=out_d[:, :], in_=res[:])

    split_excess_waits(nc)
    return nc


_NC_CACHE = None


def _get_nc():
    global _NC_CACHE
    if _NC_CACHE is None:
        _NC_CACHE = build_nc()
    return _NC_CACHE


def run_spmd(inputs, trace=False, **kw):
    feats = np.ascontiguousarray(inputs["features"], dtype=np.float32)
    n1 = np.ascontiguousarray(inputs["noise1"], dtype=np.float32)
    n2 = np.ascontiguousarray(inputs["noise2"], dtype=np.float32)
    w1 = np.ascontiguousarray(inputs["W1"], dtype=np.float32)
    b1 = np.ascontiguousarray(inputs["b1"], dtype=np.float32).reshape(D_PROJ, 1)
    w2 = np.ascontiguousarray(inputs["W2"], dtype=np.float32)
    b2 = np.ascontiguousarray(inputs["b2"], dtype=np.float32).reshape(D_PROJ, 1)

    in_maps = []
    for r in range(N_CORES):
        sl = slice(r * ROWS, (r + 1) * ROWS)
        in_maps.append({
            "features": feats[sl], "noise1": n1[sl], "noise2": n2[sl],
            "W1": w1, "b1": b1, "W2": w2, "b2": b2,
        })
    nc = _get_nc()
    return run_bass_kernel_spmd(nc, in_maps, core_ids=list(range(N_CORES)),
                                trace=trace, **kw)


def combine(results) -> np.ndarray:
    total = sum(float(results[r]["out"][0, 0]) for r in range(N_CORES))
    loss = total / float(N) - INV_T + float(np.log(np.float32(2.0)))
    return np.array(loss, dtype=np.float32)


def kernel(**inputs) -> np.ndarray:
    out = run_spmd(inputs)
    return combine(out.results)


# revision 23
# speedup vs baseline: 1.5300x; 1.3565x over previous
"""Distributed Trainium2 (Bass/Tile) kernel for the KPCL contrastive loss.

Math (matches the jax reference):
  x1 = f + sign(f) * normalize(n1, 1e-8) * 0.1
  x2 = x1 + sign(x1) * normalize(n2, 1e-8) * 0.1
     = sign(f) * (|f| + 0.1*n1/||n1|| + 0.1*n2/||n2||)
  p  = relu(x2 @ W1 + b1) @ W2 + b2
  z  = p / ||p||                          (so diag(sim) == 1/T exactly)
  sim = z @ z_all.T / T ;  lse_i = log(sum_j exp(sim_ij))
  loss = mean(lse) - 1/T + log(2)

Sharding: rows (N=8192) split across 8 cores, 1024 rows each. Each core
computes its z block in transposed layout zT [128, 1024] (bf16), AllGathers
zT to [1024, 1024] bf16, then computes its row-block of sim as bf16 128x512
matmuls (K=128 contraction). The exp+row-sum over the [1024, 8192] sim block
is split across three engines: the scalar engine runs native Exp with
accumulate, while DVE and GpSimd run a Schraudolph-style exp (bf16 bit
pattern = trunc(sim*1231.07 + 16249)) followed by an accumulate pass.
Per-core output is the scalar sum_i log(sumexp_i); the host computes
loss = sum/N - 1/T + log 2.
"""

import sys

for _p in ("/opt/trn_rl_repo",):
    if _p not in sys.path:
        sys.path.append(_p)

import numpy as np

import concourse.bass as bass
import concourse.tile as tile
from concourse import mybir
from concourse.bass_utils import run_bass_kernel_spmd
from concourse.masks import make_identity

F32 = mybir.dt.float32
F32R = mybir.dt.float32r
BF16 = mybir.dt.bfloat16
U16 = mybir.dt.uint16
U32 = mybir.dt.uint32

N_CORES = 8
N = 8192
ROWS = N // N_CORES          # 1024 rows per core
D_IN = 512
D_PROJ = 128
TEMP = 0.15
P = 128                      # partitions
NBLK = ROWS // P             # 8 row-blocks per core
INV_T = 1.0 / TEMP

# Schraudolph bf16-bit exp: bits16(e^x) ~= trunc(x * 128/ln2 + (127*128 - C))
SCH_A = 128.0 / np.log(2.0)          # 184.664965
SCH_C = 7.0                          # tuned for zero-mean sum error (trunc)
SCH_B = 127.0 * 128.0 - SCH_C

AF = mybir.ActivationFunctionType
OP = mybir.AluOpType


def split_excess_waits(nc: bass.Bass, max_waits: int = 1) -> int:
    """Hoist excess sem waits onto same-engine nop carriers.

    The walrus build in this image rejects instructions carrying more
    than ~2 sync commands ("Too many sync wait commands"), but Tile's
    wait assignment freely emits 2-3 waits per instruction. Splitting
    the waits onto preceding nop instructions on the same engine queue
    is semantically identical (engine program order is preserved).
    """
    nmoved = 0
    for f in nc.m.functions:
        for b in f.blocks:
            il = b.instructions
            i = 0
            while i < len(il):
                inst = il[i]
                si = inst.sync_info
                if si is None or not si.on_wait or len(si.on_wait) <= max_waits:
                    i += 1
                    continue
                eng = inst.engine
                if eng is None:
                    i += 1
                    continue
                waits = list(si.on_wait)
                keep = waits[-max_waits:]
                excess = waits[:-max_waits]
                carriers = []
                for w in excess:
                    nop = nc.engines[eng].nop().ins
                    for f2 in nc.m.functions:
                        for b2 in f2.blocks:
                            try:
                                b2.instructions.remove(nop)
                            except ValueError:
                                pass
                    nop.sync_info = mybir.SyncInfo(on_wait=[w], on_update=[])
                    carriers.append(nop)
                inst.sync_info = mybir.SyncInfo(on_wait=keep,
                                                on_update=list(si.on_update))
                for c in reversed(carriers):
                    il.insert(i, c)
                i += 1 + len(carriers)
                nmoved += len(excess)
    return nmoved


def build_nc(phase: str = "full") -> bass.Bass:
    # phase: "A" (local z only), "AG" (+allgather+loads), "full"
    nc = bass.Bass("TRN2", target_bir_lowering=False, debug=False,
                   num_devices=N_CORES)

    f_d = nc.dram_tensor("features", [ROWS, D_IN], F32, kind="ExternalInput")
    u1_d = nc.dram_tensor("noise1", [ROWS, D_IN], F32, kind="ExternalInput")
    u2_d = nc.dram_tensor("noise2", [ROWS, D_IN], F32, kind="ExternalInput")
    w1_d = nc.dram_tensor("W1", [D_IN, D_PROJ], F32, kind="ExternalInput")
    b1_d = nc.dram_tensor("b1", [D_PROJ, 1], F32, kind="ExternalInput")
    w2_d = nc.dram_tensor("W2", [D_PROJ, D_PROJ], F32, kind="ExternalInput")
    b2_d = nc.dram_tensor("b2", [D_PROJ, 1], F32, kind="ExternalInput")
    out_d = nc.dram_tensor("out", [1, 1], F32, kind="ExternalOutput")

    # collective bounce buffers (internal DRAM; AG output must be Shared);
    # the AllGather is chunked per phase-A group so it overlaps compute
    zT_bounce = [nc.dram_tensor(f"zT_bounce{g}", [P, ROWS // 2], BF16)
                 for g in range(2)]
    zall_bounce = [nc.dram_tensor(f"zall_bounce{g}", [N_CORES * P, ROWS // 2],
                                  BF16, addr_space="Shared")
                  for g in range(2)]

    GROUPS = 2                   # phase-A row groups
    GR = ROWS // GROUPS          # 512 rows per group
    SUB = GR // P                # 4 subtiles of 128 rows per group
    LN01 = float(np.log(0.1))    # bias so exp(-0.5*ln(s) + LN01) = 0.1/sqrt(s)

    with tile.TileContext(nc) as tc:
        with (
            tc.tile_pool(name="singles", bufs=1) as singles,
            tc.tile_pool(name="work", bufs=5) as work,
            tc.tile_pool(name="small", bufs=4) as small,
            tc.tile_pool(name="expsc", bufs=3) as expsc,
        ):
            # ---- constants / persistent tiles ----
            w1t = singles.tile([P, 4, P], F32)      # W1 k-chunks (lhsT)
            for c in range(4):
                nc.sync.dma_start(w1t[:, c, :], w1_d[c * P:(c + 1) * P, :])
            w2t = singles.tile([P, P], F32)
            nc.sync.dma_start(w2t[:], w2_d[:, :])
            b1t = singles.tile([P, 1], F32)
            nc.sync.dma_start(b1t[:], b1_d[:, :])
            b2t = singles.tile([P, 1], F32)
            nc.sync.dma_start(b2t[:], b2_d[:, :])

            ident = singles.tile([P, P], BF16)
            make_identity(nc, ident[:])
            ones_col = singles.tile([P, 1], F32)
            nc.gpsimd.memset(ones_col[:], 1.0)
            ones_col_b = singles.tile([P, 1], BF16)
            nc.gpsimd.memset(ones_col_b[:], 1.0)
            ones_row = singles.tile([1, P], F32)
            nc.gpsimd.memset(ones_row[:], 1.0)
            zbias = singles.tile([P, 1], F32)
            nc.gpsimd.memset(zbias[:], 0.0)
            ln01b = singles.tile([P, 1], F32)
            nc.gpsimd.memset(ln01b[:], float(np.log(0.1)))

            # bf16 copies of the weights for 1-cycle/row matmuls
            w1b = singles.tile([P, 4, P], BF16)
            nc.vector.tensor_copy(w1b[:], w1t[:])
            w2b = singles.tile([P, P], BF16)
            nc.vector.tensor_copy(w2b[:], w2t[:])

            zTb = singles.tile([P, ROWS], BF16)     # z^T (bf16) for this core
            logS = singles.tile([P, NBLK], F32)     # log(sumexp) per block
            zallTb = singles.tile([P, N_CORES, ROWS], BF16)  # gathered z^T

            # =========== Phase A: augment + projection + normalize ==========
            # x2 = f + sign(f) * (u1*r1 + u2*r2), r = 0.1/||u||
            # Engine split per subtile: scalar{sq1+accum, sign}; DVE{sq2+accum,
            # t1, t2, xT copy}; gpsimd{t*s mult, +f add}; PE{transposes}.
            # All activation funcs live in the natural_log_exp table (no
            # sqrt!): 0.1/sqrt(s) = exp(-0.5*ln(s) + ln(0.1)).
            with (
                tc.tile_pool(name="psA2", bufs=2, space="PSUM") as psA2,
                tc.tile_pool(name="psA1", bufs=1, space="PSUM") as psA1,
            ):
                for g in range(GROUPS):
                    xT = work.tile([P, 4, GR], BF16, tag="xT", bufs=2)
                    for half in range(SUB // 2):
                      subs = [2 * half, 2 * half + 1]
                      s1g = small.tile([P, 2], F32, tag="s1g")
                      s2g = small.tile([P, 2], F32, tag="s2g")
                      x2bs = []
                      for i, s in enumerate(subs):
                        rs = slice(g * GR + s * P, g * GR + (s + 1) * P)
                        ft = work.tile([P, D_IN], F32, tag="F")
                        nc.sync.dma_start(ft[:], f_d[rs, :])
                        u1 = work.tile([P, D_IN], F32, tag="U1")
                        nc.sync.dma_start(u1[:], u1_d[rs, :])
                        u2 = work.tile([P, D_IN], F32, tag="U2")
                        nc.sync.dma_start(u2[:], u2_d[rs, :])

                        # noise sumsq: s1 on scalar engine, s2 on DVE
                        sq1 = work.tile([P, D_IN], F32, tag="sq1")
                        nc.scalar.activation(sq1[:], u1[:], AF.Square,
                                             accum_out=s1g[:, i:i + 1])
                        sq2 = work.tile([P, D_IN], F32, tag="sq2")
                        nc.vector.scalar_tensor_tensor(
                            out=sq2[:], in0=u2[:], scalar=1.0, in1=u2[:],
                            op0=OP.mult, op1=OP.mult,
                            accum_out=s2g[:, i:i + 1])

                        # sign(f) as +-1/0 floats (scalar engine)
                        sgn = work.tile([P, D_IN], F32, tag="sgn")
                        nc.scalar.activation(sgn[:], ft[:], AF.Sign)
                        x2bs.append((ft, u1, u2, sgn))

                      # r1/r2 for both subtiles in two ln+exp pairs
                      l1 = small.tile([P, 2], F32, tag="l1")
                      nc.scalar.activation(l1[:], s1g[:], AF.Ln, bias=zbias[:])
                      r1g = small.tile([P, 2], F32, tag="r1g")
                      nc.scalar.activation(r1g[:], l1[:], AF.Exp, scale=-0.5,
                                           bias=ln01b[:])
                      l2 = small.tile([P, 2], F32, tag="l2")
                      nc.scalar.activation(l2[:], s2g[:], AF.Ln, bias=zbias[:])
                      r2g = small.tile([P, 2], F32, tag="r2g")
                      nc.scalar.activation(r2g[:], l2[:], AF.Exp, scale=-0.5,
                                           bias=ln01b[:])

                      for i, s in enumerate(subs):
                        ft, u1, u2, sgn = x2bs[i]
                        # t = u1*r1 + u2*r2 (DVE), then x2 = f + sign(f)*t
                        # (gpsimd, float ops only)
                        t1 = work.tile([P, D_IN], F32, tag="t1")
                        nc.scalar.activation(t1[:], u1[:], AF.Copy,
                                             scale=r1g[:, i:i + 1])
                        t2 = work.tile([P, D_IN], F32, tag="t2")
                        nc.vector.scalar_tensor_tensor(
                            out=t2[:], in0=u2[:], scalar=r2g[:, i:i + 1],
                            in1=t1[:], op0=OP.mult, op1=OP.add)
                        st = work.tile([P, D_IN], F32, tag="st")
                        nc.vector.tensor_tensor(out=st[:], in0=t2[:],
                                                in1=sgn[:], op=OP.mult)
                        x2b = work.tile([P, D_IN], BF16, tag="x2b")
                        nc.gpsimd.tensor_tensor(out=x2b[:], in0=st[:],
                                                in1=ft[:], op=OP.add)

                        # transpose x2b into xT[:, c, s*128:(s+1)*128]
                        tpp = psA2.tile([P, 4, P], BF16, tag="tp")
                        for c in range(4):
                            nc.tensor.transpose(tpp[:, c, :],
                                                x2b[:, c * P:(c + 1) * P],
                                                ident[:])
                        nc.vector.tensor_copy(xT[:, :, s * P:(s + 1) * P],
                                              tpp[:])

                    # hT = relu(W1^T-chunks contraction + b1)  [128, 512]
                    hps = psA1.tile([P, GR], F32, tag="hT")
                    for c in range(4):
                        nc.tensor.matmul(hps[:], w1b[:, c, :], xT[:, c, :],
                                         start=(c == 0), stop=(c == 3))
                    hT = work.tile([P, GR], BF16, tag="hT_sb")
                    nc.scalar.activation(hT[:], hps[:], AF.Relu, bias=b1t[:])

                    # pT = W2^T @ hT + b2
                    pps = psA1.tile([P, GR], F32, tag="pT")
                    nc.tensor.matmul(pps[:], w2b[:], hT[:])
                    pT = work.tile([P, GR], F32, tag="pT_sb")
                    nc.scalar.activation(pT[:], pps[:], AF.Identity,
                                         bias=b2t[:])

                    # row norms: sumsq via ones-matmul (partition reduction)
                    sqp = work.tile([P, GR], BF16, tag="sqp")
                    nc.scalar.activation(sqp[:], pT[:], AF.Square)
                    nsq = psA1.tile([1, GR], F32, tag="nsq")
                    nc.tensor.matmul(nsq[:], ones_col_b[:], sqp[:])
                    # rsz = 1/sqrt(nsq) = exp(-0.5*ln(nsq)), scalar engine
                    lnn = small.tile([1, GR], F32, tag="lnn")
                    nc.scalar.activation(lnn[:], nsq[:], AF.Ln,
                                         bias=zbias[0:1, :])
                    rsz = small.tile([1, GR], F32, tag="rsz")
                    nc.scalar.activation(rsz[:], lnn[:], AF.Exp, scale=-0.5,
                                         bias=zbias[0:1, :])

                    # broadcast rsz across partitions via K=1 matmul
                    bc = psA1.tile([P, GR], F32, tag="bc")
                    nc.tensor.matmul(bc[:], ones_row[:], rsz[:])
                    nc.vector.tensor_tensor(
                        out=zTb[:, g * GR:(g + 1) * GR], in0=pT[:], in1=bc[:],
                        op=OP.mult)

                    if phase in ("AG", "full"):
                        # AllGather this group's zT slice while the next
                        # group computes
                        nc.sync.dma_start(out=zT_bounce[g][:, :],
                                          in_=zTb[:, g * GR:(g + 1) * GR])
                        nc.gpsimd.collective_compute(
                            "AllGather",
                            OP.bypass,
                            ins=[zT_bounce[g][:, :]],
                            outs=[zall_bounce[g][:, :]],
                            replica_groups=[list(range(N_CORES))],
                        )
                        for r in range(N_CORES):
                            nc.sync.dma_start(
                                out=zallTb[:, r, g * GR:(g + 1) * GR],
                                in_=zall_bounce[g][r * P:(r + 1) * P, :])

            if phase == "A":
                dbg = small.tile([1, 1], F32, tag="dbg")
                nc.vector.tensor_copy(dbg[:], zTb[0:1, 0:1])
                nc.sync.dma_start(out=out_d[:, :], in_=dbg[:])

            if phase == "AG":
                dbg = small.tile([1, 1], F32, tag="dbg")
                nc.vector.tensor_copy(dbg[:], zallTb[0:1, 0, 0:1])
                nc.sync.dma_start(out=out_d[:, :], in_=dbg[:])

            if phase == "full":
                # ======== Phase C: sim row-block + 2-engine exp/rowsum ======
                # 4 chunks of 2048 columns per block; scalar does Exp+accum
                # on ~2.5 chunks, DVE schraudolph on ~1.5.
                with tc.tile_pool(name="psC", bufs=2, space="PSUM") as psC:
                    for m in range(NBLK):
                        lhsT = zTb[:, m * P:(m + 1) * P]
                        n_s = 3                   # chunks on scalar engine
                        sacc = small.tile([P, 4], F32, tag="sacc")
                        for ch in range(4):
                            ps = psC.tile([P, 4, 512], F32, tag="sim")
                            for j in range(4):
                                col = ch * 2048 + j * 512
                                r, off = divmod(col, ROWS)
                                nc.tensor.matmul(
                                    ps[:, j, :], lhsT,
                                    zallTb[:, r, off:off + 512])
                            acc = sacc[:, ch:ch + 1]
                            if ch < n_s:
                                sc = expsc.tile([P, 4, 512], BF16, tag="esc")
                                nc.scalar.activation(sc[:], ps[:], AF.Exp,
                                                     bias=zbias[:],
                                                     scale=INV_T,
                                                     accum_out=acc)
                            else:
                                # schraudolph: u16 = trunc(sim/T * A + B) are
                                # the bf16 bits of exp(sim/T)
                                u16t = expsc.tile([P, 4, 512], U16, tag="u16")
                                nc.vector.tensor_scalar(
                                    out=u16t[:], in0=ps[:],
                                    scalar1=float(SCH_A * INV_T),
                                    scalar2=float(SCH_B),
                                    op0=OP.mult, op1=OP.add)
                                if m < 4:
                                    dummy = expsc.tile([P, 4, 512], BF16,
                                                       tag="dm")
                                    nc.vector.tensor_scalar(
                                        out=dummy[:],
                                        in0=u16t[:].bitcast(BF16),
                                        scalar1=1.0, scalar2=0.0, op0=OP.mult,
                                        op1=OP.add, accum_out=acc)
                                else:
                                    nc.vector.tensor_reduce(
                                        out=acc, in_=u16t[:].bitcast(BF16),
                                        axis=mybir.AxisListType.XY,
                                        op=OP.add)
                        S = small.tile([P, 1], F32, tag="S")
                        nc.vector.tensor_reduce(out=S[:], in_=sacc[:],
                                                axis=mybir.AxisListType.X,
                                                op=OP.add)
                        nc.scalar.activation(logS[:, m:m + 1], S[:], AF.Ln,
                                             bias=zbias[:])

                # final local reduction: out = sum(logS)
                with tc.tile_pool(name="psF", bufs=1, space="PSUM") as psF:
                    lsum = small.tile([P, 1], F32, tag="lsum")
                    nc.vector.tensor_reduce(out=lsum[:], in_=logS[:],
                                            axis=mybir.AxisListType.X,
                                            op=OP.add)
                    lps = psF.tile([1, 1], F32, tag="lps")
                    nc.tensor.matmul(lps[:], ones_col[:], lsum[:])
                    res = small.tile([1, 1], F32, tag="res")
                    nc.vector.tensor_copy(res[:], lps[:])
                    nc.sync.dma_start(out=out_d[:, :], in_=res[:])

    split_excess_waits(nc)
    return nc


_NC_CACHE = None


def _get_nc():
    global _NC_CACHE
    if _NC_CACHE is None:
        _NC_CACHE = build_nc()
    return _NC_CACHE


def run_spmd(inputs, trace=False, **kw):
    feats = np.ascontiguousarray(inputs["features"], dtype=np.float32)
    n1 = np.ascontiguousarray(inputs["noise1"], dtype=np.float32)
    n2 = np.ascontiguousarray(inputs["noise2"], dtype=np.float32)
    w1 = np.ascontiguousarray(inputs["W1"], dtype=np.float32)
    b1 = np.ascontiguousarray(inputs["b1"], dtype=np.float32).reshape(D_PROJ, 1)
    w2 = np.ascontiguousarray(inputs["W2"], dtype=np.float32)
    b2 = np.ascontiguousarray(inputs["b2"], dtype=np.float32).reshape(D_PROJ, 1)

    in_maps = []
    for r in range(N_CORES):
        sl = slice(r * ROWS, (r + 1) * ROWS)
        in_maps.append({
            "features": feats[sl], "noise1": n1[sl], "noise2": n2[sl],
            "W1": w1, "b1": b1, "W2": w2, "b2": b2,
        })
    nc = _get_nc()
    return run_bass_kernel_spmd(nc, in_maps, core_ids=list(range(N_CORES)),
                                trace=trace, **kw)


def combine(results) -> np.ndarray:
    total = sum(float(results[r]["out"][0, 0]) for r in range(N_CORES))
    loss = total / float(N) - INV_T + float(np.log(np.float32(2.0)))
    return np.array(loss, dtype=np.float32)


def kernel(**inputs) -> np.ndarray:
    out = run_spmd(inputs)
    return combine(out.results)


# revision 25
# speedup vs baseline: 1.8840x; 1.2314x over previous
"""Distributed Trainium2 (Bass/Tile) kernel for the KPCL contrastive loss.

Math (matches the jax reference):
  x1 = f + sign(f) * normalize(n1, 1e-8) * 0.1
  x2 = x1 + sign(x1) * normalize(n2, 1e-8) * 0.1
     = sign(f) * (|f| + 0.1*n1/||n1|| + 0.1*n2/||n2||)
  p  = relu(x2 @ W1 + b1) @ W2 + b2
  z  = p / ||p||                          (so diag(sim) == 1/T exactly)
  sim = z @ z_all.T / T ;  lse_i = log(sum_j exp(sim_ij))
  loss = mean(lse) - 1/T + log(2)

Sharding: rows (N=8192) split across 8 cores, 1024 rows each. Each core
computes its z block in transposed layout zT [128, 1024] (bf16), AllGathers
zT to [1024, 1024] bf16, then computes its row-block of sim as bf16 128x512
matmuls (K=128 contraction). The exp+row-sum over the [1024, 8192] sim block
is split across three engines: the scalar engine runs native Exp with
accumulate, while DVE and GpSimd run a Schraudolph-style exp (bf16 bit
pattern = trunc(sim*1231.07 + 16249)) followed by an accumulate pass.
Per-core output is the scalar sum_i log(sumexp_i); the host computes
loss = sum/N - 1/T + log 2.
"""

import sys

for _p in ("/opt/trn_rl_repo",):
    if _p not in sys.path:
        sys.path.append(_p)

import numpy as np

import concourse.bass as bass
import concourse.tile as tile
from concourse import mybir
from concourse.bass_utils import run_bass_kernel_spmd
from concourse.masks import make_identity

F32 = mybir.dt.float32
F32R = mybir.dt.float32r
BF16 = mybir.dt.bfloat16
U16 = mybir.dt.uint16
U32 = mybir.dt.uint32

N_CORES = 8
N = 8192
ROWS = N // N_CORES          # 1024 rows per core
D_IN = 512
D_PROJ = 128
TEMP = 0.15
P = 128                      # partitions
NBLK = ROWS // P             # 8 row-blocks per core
INV_T = 1.0 / TEMP

# Schraudolph bf16-bit exp: bits16(e^x) ~= trunc(x * 128/ln2 + (127*128 - C))
SCH_A = 128.0 / np.log(2.0)          # 184.664965
SCH_C = 7.0                          # tuned for zero-mean sum error (trunc)
SCH_B = 127.0 * 128.0 - SCH_C

AF = mybir.ActivationFunctionType
OP = mybir.AluOpType


def split_excess_waits(nc: bass.Bass, max_waits: int = 1) -> int:
    """Hoist excess sem waits onto same-engine nop carriers.

    The walrus build in this image rejects instructions carrying more
    than ~2 sync commands ("Too many sync wait commands"), but Tile's
    wait assignment freely emits 2-3 waits per instruction. Splitting
    the waits onto preceding nop instructions on the same engine queue
    is semantically identical (engine program order is preserved).
    """
    nmoved = 0
    for f in nc.m.functions:
        for b in f.blocks:
            il = b.instructions
            i = 0
            while i < len(il):
                inst = il[i]
                si = inst.sync_info
                if si is None or not si.on_wait or len(si.on_wait) <= max_waits:
                    i += 1
                    continue
                eng = inst.engine
                if eng is None:
                    i += 1
                    continue
                waits = list(si.on_wait)
                keep = waits[-max_waits:]
                excess = waits[:-max_waits]
                carriers = []
                for w in excess:
                    nop = nc.engines[eng].nop().ins
                    for f2 in nc.m.functions:
                        for b2 in f2.blocks:
                            try:
                                b2.instructions.remove(nop)
                            except ValueError:
                                pass
                    nop.sync_info = mybir.SyncInfo(on_wait=[w], on_update=[])
                    carriers.append(nop)
                inst.sync_info = mybir.SyncInfo(on_wait=keep,
                                                on_update=list(si.on_update))
                for c in reversed(carriers):
                    il.insert(i, c)
                i += 1 + len(carriers)
                nmoved += len(excess)
    return nmoved


def build_nc(phase: str = "full") -> bass.Bass:
    # phase: "A" (local z only), "full"
    nc = bass.Bass("TRN2", target_bir_lowering=False, debug=False,
                   num_devices=N_CORES)

    f_d = nc.dram_tensor("features", [ROWS, D_IN], F32, kind="ExternalInput")
    u1_d = nc.dram_tensor("noise1", [ROWS, D_IN], F32, kind="ExternalInput")
    u2_d = nc.dram_tensor("noise2", [ROWS, D_IN], F32, kind="ExternalInput")
    w1_d = nc.dram_tensor("W1", [D_IN, D_PROJ], F32, kind="ExternalInput")
    b1_d = nc.dram_tensor("b1", [D_PROJ, 1], F32, kind="ExternalInput")
    w2_d = nc.dram_tensor("W2", [D_PROJ, D_PROJ], F32, kind="ExternalInput")
    b2_d = nc.dram_tensor("b2", [D_PROJ, 1], F32, kind="ExternalInput")
    out_d = nc.dram_tensor("out", [1, 1], F32, kind="ExternalOutput")

    GROUPS = 4                   # phase-A row groups == AllGather chunks
    GR = ROWS // GROUPS          # 256 rows per group
    SUB = GR // P                # 2 subtiles of 128 rows per group

    # collective bounce buffers (internal DRAM; AG output must be Shared);
    # the AllGather is chunked per phase-A group so it overlaps compute
    zT_bounce = [nc.dram_tensor(f"zT_bounce{g}", [P, GR], BF16)
                 for g in range(GROUPS)]
    zall_bounce = [nc.dram_tensor(f"zall_bounce{g}", [N_CORES * P, GR],
                                  BF16, addr_space="Shared")
                   for g in range(GROUPS)]

    with tile.TileContext(nc) as tc:
        with (
            tc.tile_pool(name="singles", bufs=1) as singles,
            tc.tile_pool(name="work", bufs=5) as work,
            tc.tile_pool(name="small", bufs=4) as small,
            tc.tile_pool(name="expsc", bufs=3) as expsc,
        ):
            # ---- constants / persistent tiles ----
            w1t = singles.tile([P, 4, P], F32)      # W1 k-chunks (lhsT)
            for c in range(4):
                nc.sync.dma_start(w1t[:, c, :], w1_d[c * P:(c + 1) * P, :])
            w2t = singles.tile([P, P], F32)
            nc.sync.dma_start(w2t[:], w2_d[:, :])
            b1t = singles.tile([P, 1], F32)
            nc.sync.dma_start(b1t[:], b1_d[:, :])
            b2t = singles.tile([P, 1], F32)
            nc.sync.dma_start(b2t[:], b2_d[:, :])

            ident = singles.tile([P, P], BF16)
            make_identity(nc, ident[:])
            ones_col = singles.tile([P, 1], F32)
            nc.gpsimd.memset(ones_col[:], 1.0)
            ones_col_b = singles.tile([P, 1], BF16)
            nc.gpsimd.memset(ones_col_b[:], 1.0)
            ones_row = singles.tile([1, P], F32)
            nc.gpsimd.memset(ones_row[:], 1.0)
            zbias = singles.tile([P, 1], F32)
            nc.gpsimd.memset(zbias[:], 0.0)
            ln01b = singles.tile([P, 1], F32)
            nc.gpsimd.memset(ln01b[:], float(np.log(0.1)))

            # bf16 copies of the weights for 1-cycle/row matmuls
            w1b = singles.tile([P, 4, P], BF16)
            nc.vector.tensor_copy(w1b[:], w1t[:])
            w2b = singles.tile([P, P], BF16)
            nc.vector.tensor_copy(w2b[:], w2t[:])

            zTb = singles.tile([P, ROWS], BF16)     # z^T (bf16) for this core
            logS = singles.tile([P, NBLK], F32)     # log(sumexp) per block
            saccA = singles.tile([P, NBLK, 4], F32)  # exp-sum partials
            zallTb = singles.tile([P, N_CORES, ROWS], BF16)  # gathered z^T

            # =========== Phase A: augment + projection + normalize ==========
            # x2 = f + sign(f) * (u1*r1 + u2*r2), r = 0.1/||u||
            # All activation funcs live in the natural_log_exp table (no
            # sqrt!): 0.1/sqrt(s) = exp(-0.5*ln(s) + ln(0.1)).
            with (
                tc.tile_pool(name="psA2", bufs=2, space="PSUM") as psA2,
                tc.tile_pool(name="psA1", bufs=1, space="PSUM") as psA1,
            ):
                for g in range(GROUPS):
                    xT = work.tile([P, 4, GR], BF16, tag="xT", bufs=2)
                    s1g = small.tile([P, SUB], F32, tag="s1g")
                    s2g = small.tile([P, SUB], F32, tag="s2g")
                    held = []
                    for i in range(SUB):
                        rs = slice(g * GR + i * P, g * GR + (i + 1) * P)
                        ft = work.tile([P, D_IN], F32, tag="F", bufs=6)
                        nc.sync.dma_start(ft[:], f_d[rs, :])
                        u1 = work.tile([P, D_IN], F32, tag="U1", bufs=6)
                        nc.sync.dma_start(u1[:], u1_d[rs, :])
                        u2 = work.tile([P, D_IN], F32, tag="U2", bufs=6)
                        nc.sync.dma_start(u2[:], u2_d[rs, :])

                        # noise sumsq: s1 on scalar engine, s2 on DVE
                        sq1 = work.tile([P, D_IN], F32, tag="sq1")
                        nc.scalar.activation(sq1[:], u1[:], AF.Square,
                                             accum_out=s1g[:, i:i + 1])
                        sq2 = work.tile([P, D_IN], F32, tag="sq2")
                        nc.vector.scalar_tensor_tensor(
                            out=sq2[:], in0=u2[:], scalar=1.0, in1=u2[:],
                            op0=OP.mult, op1=OP.mult,
                            accum_out=s2g[:, i:i + 1])

                        # sign(f) as +-1/0 floats (scalar engine)
                        sgn = work.tile([P, D_IN], F32, tag="sgn")
                        nc.scalar.activation(sgn[:], ft[:], AF.Sign)
                        held.append((ft, u1, u2, sgn))

                    # r1/r2 for the group in two ln+exp pairs
                    l1 = small.tile([P, SUB], F32, tag="l1")
                    nc.scalar.activation(l1[:], s1g[:], AF.Ln, bias=zbias[:])
                    r1g = small.tile([P, SUB], F32, tag="r1g")
                    nc.scalar.activation(r1g[:], l1[:], AF.Exp, scale=-0.5,
                                         bias=ln01b[:])
                    l2 = small.tile([P, SUB], F32, tag="l2")
                    nc.scalar.activation(l2[:], s2g[:], AF.Ln, bias=zbias[:])
                    r2g = small.tile([P, SUB], F32, tag="r2g")
                    nc.scalar.activation(r2g[:], l2[:], AF.Exp, scale=-0.5,
                                         bias=ln01b[:])

                    for i in range(SUB):
                        ft, u1, u2, sgn = held[i]
                        # t = u1*r1 + u2*r2, x2 = f + sign(f)*t
                        t1 = work.tile([P, D_IN], F32, tag="t1")
                        nc.scalar.activation(t1[:], u1[:], AF.Copy,
                                             scale=r1g[:, i:i + 1])
                        t2 = work.tile([P, D_IN], F32, tag="t2")
                        nc.vector.scalar_tensor_tensor(
                            out=t2[:], in0=u2[:], scalar=r2g[:, i:i + 1],
                            in1=t1[:], op0=OP.mult, op1=OP.add)
                        st = work.tile([P, D_IN], F32, tag="st")
                        nc.vector.tensor_tensor(out=st[:], in0=t2[:],
                                                in1=sgn[:], op=OP.mult)
                        x2b = work.tile([P, D_IN], BF16, tag="x2b")
                        nc.gpsimd.tensor_tensor(out=x2b[:], in0=st[:],
                                                in1=ft[:], op=OP.add)

                        # transpose x2b into xT[:, c, i*128:(i+1)*128]
                        tpp = psA2.tile([P, 4, P], BF16, tag="tp")
                        for c in range(4):
                            nc.tensor.transpose(tpp[:, c, :],
                                                x2b[:, c * P:(c + 1) * P],
                                                ident[:])
                        nc.vector.tensor_copy(xT[:, :, i * P:(i + 1) * P],
                                              tpp[:])

                    # hT = relu(W1^T-chunks contraction + b1)  [128, GR]
                    hps = psA1.tile([P, GR], F32, tag="hT")
                    for c in range(4):
                        nc.tensor.matmul(hps[:], w1b[:, c, :], xT[:, c, :],
                                         start=(c == 0), stop=(c == 3))
                    hT = work.tile([P, GR], BF16, tag="hT_sb")
                    nc.scalar.activation(hT[:], hps[:], AF.Relu, bias=b1t[:])

                    # pT = W2^T @ hT + b2
                    pps = psA1.tile([P, GR], F32, tag="pT")
                    nc.tensor.matmul(pps[:], w2b[:], hT[:])
                    pT = work.tile([P, GR], F32, tag="pT_sb")
                    nc.scalar.activation(pT[:], pps[:], AF.Identity,
                                         bias=b2t[:])

                    # row norms: sumsq via ones-matmul (partition reduction)
                    sqp = work.tile([P, GR], BF16, tag="sqp")
                    nc.scalar.activation(sqp[:], pT[:], AF.Square)
                    nsq = psA1.tile([1, GR], F32, tag="nsq")
                    nc.tensor.matmul(nsq[:], ones_col_b[:], sqp[:])
                    # rsz = 1/sqrt(nsq) = exp(-0.5*ln(nsq)), scalar engine
                    lnn = small.tile([1, GR], F32, tag="lnn")
                    nc.scalar.activation(lnn[:], nsq[:], AF.Ln,
                                         bias=zbias[0:1, :])
                    rsz = small.tile([1, GR], F32, tag="rsz")
                    nc.scalar.activation(rsz[:], lnn[:], AF.Exp, scale=-0.5,
                                         bias=zbias[0:1, :])

                    # broadcast rsz across partitions via K=1 matmul
                    bc = psA1.tile([P, GR], F32, tag="bc")
                    nc.tensor.matmul(bc[:], ones_row[:], rsz[:])
                    nc.vector.tensor_tensor(
                        out=zTb[:, g * GR:(g + 1) * GR], in0=pT[:], in1=bc[:],
                        op=OP.mult)

                    if phase == "full":
                        # AllGather this group's zT slice while later groups
                        # compute
                        nc.sync.dma_start(out=zT_bounce[g][:, :],
                                          in_=zTb[:, g * GR:(g + 1) * GR])
                        nc.gpsimd.collective_compute(
                            "AllGather",
                            OP.bypass,
                            ins=[zT_bounce[g][:, :]],
                            outs=[zall_bounce[g][:, :]],
                            replica_groups=[list(range(N_CORES))],
                        )
                        for r in range(N_CORES):
                            nc.sync.dma_start(
                                out=zallTb[:, r, g * GR:(g + 1) * GR],
                                in_=zall_bounce[g][r * P:(r + 1) * P, :])

            if phase == "A":
                dbg = small.tile([1, 1], F32, tag="dbg")
                nc.vector.tensor_copy(dbg[:], zTb[0:1, 0:1])
                nc.sync.dma_start(out=out_d[:, :], in_=dbg[:])

            if phase == "full":
                # ======== Phase C: sweep column-quarters; sweep c only needs
                # AG chunk c, so later AG chunks hide under earlier sweeps.
                # Per (block, sweep): 8 matmuls of [128,256] = 2048 columns;
                # scalar engine Exp+accum on 3 of 4 sweeps, DVE schraudolph
                # on the rest, rotated so each sweep mixes both engines.
                with tc.tile_pool(name="psC", bufs=2, space="PSUM") as psC:
                    for c in range(GROUPS):
                        for m in range(NBLK):
                            lhsT = zTb[:, m * P:(m + 1) * P]
                            ps = psC.tile([P, 8, GR], F32, tag="sim")
                            for r in range(N_CORES):
                                nc.tensor.matmul(
                                    ps[:, r, :], lhsT,
                                    zallTb[:, r, c * GR:(c + 1) * GR])
                            acc = saccA[:, m, c:c + 1]
                            if (m + c) % 4 != 3:
                                sc = expsc.tile([P, 8, GR], BF16, tag="esc")
                                nc.scalar.activation(sc[:], ps[:], AF.Exp,
                                                     bias=zbias[:],
                                                     scale=INV_T,
                                                     accum_out=acc)
                            else:
                                # schraudolph: u16 = trunc(sim/T * A + B) are
                                # the bf16 bits of exp(sim/T)
                                u16t = expsc.tile([P, 8, GR], U16, tag="u16")
                                nc.vector.tensor_scalar(
                                    out=u16t[:], in0=ps[:],
                                    scalar1=float(SCH_A * INV_T),
                                    scalar2=float(SCH_B),
                                    op0=OP.mult, op1=OP.add)
                                nc.vector.tensor_reduce(
                                    out=acc, in_=u16t[:].bitcast(BF16),
                                    axis=mybir.AxisListType.XY,
                                    op=OP.add)
                            if c == GROUPS - 1:
                                S = small.tile([P, 1], F32, tag="S")
                                nc.vector.tensor_reduce(
                                    out=S[:], in_=saccA[:, m, :],
                                    axis=mybir.AxisListType.X, op=OP.add)
                                nc.scalar.activation(logS[:, m:m + 1], S[:],
                                                     AF.Ln, bias=zbias[:])

                # final local reduction: out = sum(logS)
                with tc.tile_pool(name="psF", bufs=1, space="PSUM") as psF:
                    lsum = small.tile([P, 1], F32, tag="lsum")
                    nc.vector.tensor_reduce(out=lsum[:], in_=logS[:],
                                            axis=mybir.AxisListType.X,
                                            op=OP.add)
                    lps = psF.tile([1, 1], F32, tag="lps")
                    nc.tensor.matmul(lps[:], ones_col[:], lsum[:])
                    res = small.tile([1, 1], F32, tag="res")
                    nc.vector.tensor_copy(res[:], lps[:])
                    nc.sync.dma_start(out=out_d[:, :], in_=res[:])

    split_excess_waits(nc)
    return nc


_NC_CACHE = None


def _get_nc():
    global _NC_CACHE
    if _NC_CACHE is None:
        _NC_CACHE = build_nc()
    return _NC_CACHE


def run_spmd(inputs, trace=False, **kw):
    feats = np.ascontiguousarray(inputs["features"], dtype=np.float32)
    n1 = np.ascontiguousarray(inputs["noise1"], dtype=np.float32)
    n2 = np.ascontiguousarray(inputs["noise2"], dtype=np.float32)
    w1 = np.ascontiguousarray(inputs["W1"], dtype=np.float32)
    b1 = np.ascontiguousarray(inputs["b1"], dtype=np.float32).reshape(D_PROJ, 1)
    w2 = np.ascontiguousarray(inputs["W2"], dtype=np.float32)
    b2 = np.ascontiguousarray(inputs["b2"], dtype=np.float32).reshape(D_PROJ, 1)

    in_maps = []
    for r in range(N_CORES):
        sl = slice(r * ROWS, (r + 1) * ROWS)
        in_maps.append({
            "features": feats[sl], "noise1": n1[sl], "noise2": n2[sl],
            "W1": w1, "b1": b1, "W2": w2, "b2": b2,
        })
    nc = _get_nc()
    return run_bass_kernel_spmd(nc, in_maps, core_ids=list(range(N_CORES)),
                                trace=trace, **kw)


def combine(results) -> np.ndarray:
    total = sum(float(results[r]["out"][0, 0]) for r in range(N_CORES))
    loss = total / float(N) - INV_T + float(np.log(np.float32(2.0)))
    return np.array(loss, dtype=np.float32)


def kernel(**inputs) -> np.ndarray:
    out = run_spmd(inputs)
    return combine(out.results)
